# revision 1
# baseline (speedup 1.0000x reference)
"""Trainium2 Bass kernel for nn_Deconv2DVariableWeights (8-core SPMD).

Strategy:
  Phase 1 (dense + BN), unit-sharded: core c owns dense_w columns
  [c*18432, (c+1)*18432) = conv input channels s in [16c, 16c+16).
  It computes h = relu(z @ w + b) for ALL 32 samples in (units, batch)
  layout, so BatchNorm batch statistics are local free-dim reductions.
  Normalized kernels are PE-transposed to (batch, units) rows and
  written to DRAM.

  AllToAll redistributes kernels: afterwards core c holds the FULL
  147456-long kernel vector for its 4 samples (batch-sharded).

  Phase 2 (conv + residual), batch-sharded: per sample, the 3x3
  conv is 9 PSUM-accumulated matmuls (K=s=128, M=f=128, N=512x2)
  against a zero-padded image tile addressed with strided APs.
"""

import numpy as np

import concourse.bacc as bacc
import concourse.bass as bass
import concourse.tile as tile
from concourse import mybir
from concourse.bass_utils import run_bass_kernel_spmd
from concourse.masks import make_identity
from concourse.tile import add_dep_helper

# Problem constants (hardcoded per contract)
B, C, H, W = 32, 128, 32, 32
ZDIM = 256
KH = KW = 3
NB = C * C * KH * KW          # 147456
N_CORES = 8
NB_L = NB // N_CORES          # 18432 units per core
B_L = B // N_CORES            # 4 samples per core
S_L = C // N_CORES            # 16 input channels per core
N_TILES = NB_L // 128         # 144 unit tiles per core
GROUP = 16                    # unit tiles per processing group
N_GROUPS = N_TILES // GROUP   # 9
BN_EPS = 1e-6
PIX = H * W                   # 1024
PAD = H + 2                   # 34

# Precision of matmul inputs (w, z) and of the generated conv kernels
# (through the AllToAll and the conv lhsT / rhs). fp16 keeps ~3.5
# significant digits vs bf16's ~2.5 at identical throughput.
W_DT = mybir.dt.float16
K_DT = mybir.dt.float16

_CACHE: dict = {}


def _np_of(dt):
    return mybir.dt.np(dt)


def _build_nc(db_zero: bool, b_zero: bool):
    nc = bacc.Bacc(num_devices=N_CORES)
    f32 = mybir.dt.float32
    GC = GROUP * 128            # columns per group in unit space (2048)
    GB = GROUP * B              # columns per group in (tile,batch) space (512)

    w_in = nc.declare_dram_parameter("w", [128, 2 * NB_L], W_DT, isOutput=False)
    zT_in = nc.declare_dram_parameter("zT", [128, 2 * B], W_DT, isOutput=False)
    db_in = nc.declare_dram_parameter("db_r", [128, N_TILES], f32, isOutput=False)
    gam_in = nc.declare_dram_parameter("gamma_r", [128, N_TILES], f32, isOutput=False)
    bet_in = nc.declare_dram_parameter("beta_r", [128, N_TILES], f32, isOutput=False)
    x_in = nc.declare_dram_parameter("x", [B_L, C, PIX], f32, isOutput=False)
    bv_in = nc.declare_dram_parameter("bvec", [C, 1], f32, isOutput=False)
    out_p = nc.declare_dram_parameter("out", [B_L, C, PIX], f32, isOutput=True)

    cc_in = nc.dram_tensor("cc_in", [B, NB_L], K_DT)
    cc_out = nc.dram_tensor("cc_out", [B, NB_L], K_DT)

    with tile.TileContext(nc) as tc:
        with tc.tile_pool(name="singles", bufs=1) as singles:
            zT = singles.tile([128, 2 * B], W_DT)
            nc.scalar.dma_start(out=zT[:, :], in_=zT_in[:, :])
            gam = singles.tile([128, N_TILES], f32)
            nc.scalar.dma_start(out=gam[:, :], in_=gam_in[:, :])
            bet = singles.tile([128, N_TILES], f32)
            nc.scalar.dma_start(out=bet[:, :], in_=bet_in[:, :])
            if not db_zero:
                db = singles.tile([128, N_TILES], f32)
                nc.scalar.dma_start(out=db[:, :], in_=db_in[:, :])
            if not b_zero:
                bv = singles.tile([C, 1], f32)
                nc.scalar.dma_start(out=bv[:, :], in_=bv_in[:, :])
            ident = singles.tile([128, 128], K_DT)
            make_identity(nc, ident)
            eps_t = singles.tile([128, 1], f32)
            nc.vector.memset(eps_t, BN_EPS)

            # ------- Phase 1: per-group dense + BN + transpose (pipelined) ----
            with (
                tc.tile_pool(name="wpool", bufs=4) as wpool,
                tc.tile_pool(name="npool", bufs=6) as npool,
                tc.tile_pool(name="stat", bufs=6) as stat,
                tc.tile_pool(name="ph", bufs=3, space="PSUM") as psum_h,
                tc.tile_pool(name="pt", bufs=3, space="PSUM") as psum_t,
            ):
                w3 = w_in.rearrange("p (k j) -> p k j", k=2)
                last_ktr = []
                for g in range(N_GROUPS):
                    gsl = slice(g * GROUP, (g + 1) * GROUP)
                    wt = wpool.tile([128, 2, GC], W_DT, tag="wt")
                    nc.sync.dma_start(
                        out=wt[:, :, :], in_=w3[:, :, g * GC:(g + 1) * GC])
                    ps = psum_h.tile([128, GB], f32, tag="ps")
                    # write position j hosts tile t(j) = (j%4)*M4 + j//4 so
                    # the 4-tile transposes below read contiguous 128-col slabs
                    M4 = GROUP // 4
                    for j in range(GROUP):
                        t_tile = (j % 4) * M4 + j // 4
                        for k in range(2):
                            nc.tensor.matmul(
                                ps[:, j * B:(j + 1) * B],
                                wt[:, k, t_tile * 128:(t_tile + 1) * 128],
                                zT[:, k * B:(k + 1) * B],
                                start=(k == 0),
                                stop=(k == 1),
                            )
                    if not db_zero:
                        nc.vector.tensor_tensor(
                            out=ps.rearrange("p (t b) -> p t b", b=B),
                            in0=ps.rearrange("p (t b) -> p t b", b=B),
                            in1=db[:, gsl].to_broadcast([128, GROUP, B]),
                            op=mybir.AluOpType.add)
                    hg = npool.tile([128, GB], f32, tag="hg")
                    nc.scalar.activation(
                        out=hg[:, :], in_=ps[:, :],
                        func=mybir.ActivationFunctionType.Relu)
                    h3 = hg.rearrange("p (t b) -> p t b", b=B)
                    s1 = stat.tile([128, GROUP], f32, tag="s1")
                    nc.vector.reduce_sum(
                        out=s1[:, :], in_=h3, axis=mybir.AxisListType.X)
                    sq = npool.tile([128, GB], f32, tag="sq")
                    nc.vector.tensor_mul(sq[:, :], hg[:, :], hg[:, :])
                    s2 = stat.tile([128, GROUP], f32, tag="s2")
                    nc.vector.reduce_sum(
                        out=s2[:, :],
                        in_=sq.rearrange("p (t b) -> p t b", b=B),
                        axis=mybir.AxisListType.X)
                    # var = (B*S2 - S1^2)/B^2; std = sqrt(q)/B
                    t1 = stat.tile([128, GROUP], f32, tag="t1")
                    nc.scalar.activation(
                        out=t1[:, :], in_=s1[:, :],
                        func=mybir.ActivationFunctionType.Square)
                    q = stat.tile([128, GROUP], f32, tag="q")
                    nc.vector.scalar_tensor_tensor(
                        out=q[:, :], in0=s2[:, :], scalar=float(B),
                        in1=t1[:, :], op0=mybir.AluOpType.mult,
                        op1=mybir.AluOpType.subtract)
                    nc.scalar.activation(
                        out=q[:, :], in_=q[:, :],
                        func=mybir.ActivationFunctionType.Sqrt,
                        scale=float(1.0 / (B * B)))
                    nc.scalar.add(out=q[:, :], in_=q[:, :], add=eps_t[:, 0:1])
                    nc.vector.reciprocal(out=q[:, :], in_=q[:, :])
                    scl = stat.tile([128, GROUP], f32, tag="scl")
                    nc.vector.tensor_mul(scl[:, :], q[:, :], gam[:, gsl])
                    t2 = stat.tile([128, GROUP], f32, tag="t2")
                    nc.vector.tensor_mul(t2[:, :], s1[:, :], scl[:, :])
                    sft = stat.tile([128, GROUP], f32, tag="sft")
                    nc.vector.scalar_tensor_tensor(
                        out=sft[:, :], in0=t2[:, :], scalar=float(-1.0 / B),
                        in1=bet[:, gsl], op0=mybir.AluOpType.mult,
                        op1=mybir.AluOpType.add)
                    # kern = h*scale + shift, cast to K_DT (on idle GpSimd)
                    tmp = npool.tile([128, GB], f32, tag="tmp")
                    nc.gpsimd.tensor_mul(
                        tmp.rearrange("p (t b) -> p t b", b=B), h3,
                        scl[:, :].to_broadcast([128, GROUP, B]))
                    kern = npool.tile([128, GB], K_DT, tag="kern")
                    nc.gpsimd.tensor_tensor(
                        out=kern.rearrange("p (t b) -> p t b", b=B),
                        in0=tmp.rearrange("p (t b) -> p t b", b=B),
                        in1=sft[:, :].to_broadcast([128, GROUP, B]),
                        op=mybir.AluOpType.add)
                    # 4 unit-tiles per PE transpose: in (128, 4x32 strided
                    # tile-cols) -> out (128=(t4,b), 128=p)
                    pt = psum_t.tile([128, GROUP // 4 * 128], K_DT, tag="pt")
                    for m in range(GROUP // 4):
                        nc.tensor.transpose(
                            pt[:, m * 128:(m + 1) * 128],
                            kern[:, m * 128:(m + 1) * 128],
                            ident[:, :],
                        )
                    ktr = npool.tile([128, GROUP // 4 * 128], K_DT, tag="ktr")
                    nc.any.tensor_copy(out=ktr[:, :], in_=pt[:, :])
                    # dest: addr = b*NB_L + g*GC + t4*(M4*128) + (m*128 + p)
                    for t4 in range(4):
                        dst = bass.AP(
                            tensor=cc_in[:, :].tensor,
                            offset=g * GC + t4 * (M4 * 128),
                            ap=[[NB_L, B], [1, M4 * 128]],
                        )
                        eng = (nc.scalar, nc.gpsimd)[(g * 4 + t4) % 2]
                        kd = eng.dma_start(
                            out=dst, in_=ktr[t4 * B:(t4 + 1) * B, :])
                        if g == N_GROUPS - 1:
                            last_ktr.append(kd)

            # ---------------- Phase 2 prep (overlaps the collective) ---------
            with (
                tc.tile_pool(name="conv", bufs=1) as conv,
                tc.tile_pool(name="conv2", bufs=2) as conv2,
                tc.tile_pool(name="po", bufs=4, space="PSUM") as psum_o,
            ):
                # ---------------- AllToAll: batch redistribute ---------------
                nc.gpsimd.collective_compute(
                    "AllToAll",
                    mybir.AluOpType.bypass,
                    replica_groups=[list(range(N_CORES))],
                    ins=[cc_in[:, :]],
                    outs=[cc_out[:, :]],
                )

                # phase-2 prep: deferred until phase-1 stores finish so it
                # fills the collective window instead of stalling the phase-1
                # tail; kept off gpsimd so the A2A issue is never queued
                # behind it on the Pool sequencer
                x_all = conv.tile([128, B_L, PIX], f32)
                xd = nc.scalar.dma_start(
                    out=x_all[:, :, :],
                    in_=x_in.rearrange("b p j -> p b j"))
                add_dep_helper(xd.ins, last_ktr[-1].ins, sync=True,
                               reason="defer x load into A2A window")
                xp_all = conv.tile([128, B_L, PAD * PAD], K_DT)
                ms = nc.vector.memset(xp_all[:, :, :], 0.0)
                add_dep_helper(ms.ins, last_ktr[-1].ins, sync=True,
                               reason="defer pad memset into A2A window")
                nc.vector.tensor_copy(
                    out=xp_all.rearrange("p b (r c) -> p b r c", c=PAD)
                    [:, :, 1:H + 1, 1:W + 1],
                    in_=x_all.rearrange("p b (r c) -> p b r c", c=W),
                )

                # ---------------- Phase 2: per-sample conv + residual --------
                hks = []
                for i in range(B_L):
                    hk = conv2.tile([128, C * KH * KW], K_DT, tag="hk")
                    if i == 0:
                        # split the first load across two engines: it gates
                        # the whole conv phase right after the AllToAll
                        for hf in range(2):
                            srcap = bass.AP(
                                tensor=cc_out[:, :].tensor,
                                offset=i * NB_L + hf * 4 * B_L * NB_L,
                                ap=[[B_L * NB_L, N_CORES // 2],
                                    [C * KH * KW, S_L], [1, C * KH * KW]],
                            )
                            (nc.scalar, nc.sync)[hf].dma_start(
                                out=hk[hf * 64:(hf + 1) * 64, :], in_=srcap)
                    else:
                        srcap = bass.AP(
                            tensor=cc_out[:, :].tensor,
                            offset=i * NB_L,
                            ap=[[B_L * NB_L, N_CORES], [C * KH * KW, S_L],
                                [1, C * KH * KW]],
                        )
                        (nc.scalar, nc.sync)[i % 2].dma_start(
                            out=hk[:, :], in_=srcap)
                    hks.append(hk)
                for i in range(B_L):
                    po = psum_o.tile([128, PIX], f32, tag="po")
                    hk9 = hks[i].rearrange("p (f n) -> p n f", n=KH * KW)
                    xp3 = xp_all[:, i, :].rearrange("p (r c) -> p r c", c=PAD)
                    ob = conv2.tile([128, PIX], f32, tag="ob")
                    last = i == B_L - 1
                    for hh in range(2):
                        for tap in range(KH * KW):
                            u, v = tap // KW, tap % KW
                            r0 = hh * 16 + u
                            nc.tensor.matmul(
                                po[:, hh * 512:(hh + 1) * 512],
                                hk9[:, tap, :],
                                xp3[:, r0:r0 + 16, v:v + W],
                                start=(tap == 0),
                                stop=(tap == KH * KW - 1),
                            )
                        if last:
                            # epilogue per half, inline: half-0's add+store
                            # overlaps half-1's taps on the PE
                            hsl = slice(hh * 512, (hh + 1) * 512)
                            nc.vector.tensor_add(
                                out=ob[:, hsl], in0=po[:, hsl],
                                in1=x_all[:, i, hsl])
                            if not b_zero:
                                nc.scalar.add(
                                    out=ob[:, hsl], in_=ob[:, hsl],
                                    add=bv[:, 0:1])
                            nc.sync.dma_start(
                                out=out_p[i, :, hsl], in_=ob[:, hsl])
                    if not last:
                        # out = conv + x (+ b)
                        nc.vector.tensor_add(
                            out=ob[:, :], in0=po[:, :], in1=x_all[:, i, :])
                        if not b_zero:
                            nc.scalar.add(
                                out=ob[:, :], in_=ob[:, :], add=bv[:, 0:1])
                        nc.sync.dma_start(
                            out=out_p[i, :, :], in_=ob[:, :])

    nc.compile()
    return nc


def _perm_groups(v144):
    # reorder unit-tile columns so position j = m*4+t4 holds tile t4*(G/4)+m
    r = v144.reshape(128, N_TILES // GROUP, 4, GROUP // 4)
    return np.ascontiguousarray(r.transpose(0, 1, 3, 2).reshape(128, N_TILES))


def _make_in_maps(x, z, dense_w, dense_b, gamma, beta, b):
    wnp = _np_of(W_DT)
    f32 = np.float32
    # zT[p, k*B + bb] = z[bb, 128k + p]
    zr = np.ascontiguousarray(z.T.astype(f32)).reshape(2, 128, B)
    zT = np.concatenate([zr[0], zr[1]], axis=1).astype(wnp)
    bvec = np.asarray(b, dtype=f32).reshape(C, 1)
    in_maps = []
    for c in range(N_CORES):
        sl = slice(c * NB_L, (c + 1) * NB_L)
        ws = dense_w[:, sl]
        w_host = np.ascontiguousarray(
            np.concatenate([ws[:128, :], ws[128:, :]], axis=1)).astype(wnp)
        in_maps.append({
            "w": w_host,
            "zT": zT,
            "db_r": _perm_groups(
                np.asarray(dense_b, dtype=f32)[sl].reshape(N_TILES, 128).T),
            "gamma_r": _perm_groups(
                np.asarray(gamma, dtype=f32)[sl].reshape(N_TILES, 128).T),
            "beta_r": _perm_groups(
                np.asarray(beta, dtype=f32)[sl].reshape(N_TILES, 128).T),
            "x": np.ascontiguousarray(
                np.asarray(x, dtype=f32)[c * B_L:(c + 1) * B_L]
                .reshape(B_L, C, PIX)),
            "bvec": bvec,
        })
    return in_maps


def kernel(x, z, dense_w, dense_b, gamma, beta, b):
    import time

    x, z, dense_w = np.asarray(x), np.asarray(z), np.asarray(dense_w)
    dense_b, gamma = np.asarray(dense_b), np.asarray(gamma)
    beta, b = np.asarray(beta), np.asarray(b)
    key = (bool(np.all(dense_b == 0)), bool(np.all(b == 0)))
    if key not in _CACHE:
        _CACHE[key] = _build_nc(*key)
        _CACHE["nc"] = _CACHE[key]
    nc = _CACHE[key]
    in_maps = _make_in_maps(x, z, dense_w, dense_b, gamma, beta, b)
    res = None
    for attempt in range(3):
        try:
            res = run_bass_kernel_spmd(nc, in_maps, list(range(N_CORES)))
            break
        except Exception:
            # transient NRT device-unrecoverable errors heal on retry
            if attempt == 2:
                raise
            time.sleep(2.0)
    out = np.concatenate(
        [res.results[c]["out"].reshape(B_L, C, H, W) for c in range(N_CORES)],
        axis=0,
    )
    return out.astype(np.float32)



# revision 8
# speedup vs baseline: 1.2057x; 1.2057x over previous
"""Trainium2 Bass kernel for nn_Deconv2DVariableWeights (8-core SPMD).

Strategy:
  Phase 1 (dense + BN), unit-sharded: core c owns dense_w columns
  [c*18432, (c+1)*18432) = conv input channels s in [16c, 16c+16).
  It computes h = relu(z @ w + b) for ALL 32 samples in (units, batch)
  layout, so BatchNorm batch statistics are local free-dim reductions.
  Normalized kernels are PE-transposed to (batch, units) rows and
  written to DRAM.

  AllToAll redistributes kernels: afterwards core c holds the FULL
  147456-long kernel vector for its 4 samples (batch-sharded).

  Phase 2 (conv + residual), batch-sharded: per sample, the 3x3
  conv is 9 PSUM-accumulated matmuls (K=s=128, M=f=128, N=512x2)
  against a zero-padded image tile addressed with strided APs.

  Dispatch-IO precision: the per-dispatch cost through the axon tunnel
  is dominated by staging the argument buffers (~360 GB/s aggregate),
  so the kernel minimizes IO bytes. dense_w ships as fp8e4 scaled by
  2^12 on the host (its values ~6e-3 sit below e4m3's normal range;
  the power-of-two scale is absorbed exactly by the BatchNorm divide,
  eps effect ~1e-5 relative). x and out ship as fp16; gamma/beta ride
  in one stacked tensor.
"""

import numpy as np

import concourse.bacc as bacc
import concourse.bass as bass
import concourse.tile as tile
from concourse import mybir
from concourse.bass_utils import run_bass_kernel_spmd
from concourse.masks import make_identity
from concourse.tile import add_dep_helper

# Problem constants (hardcoded per contract)
B, C, H, W = 32, 128, 32, 32
ZDIM = 256
KH = KW = 3
NB = C * C * KH * KW          # 147456
N_CORES = 8
NB_L = NB // N_CORES          # 18432 units per core
B_L = B // N_CORES            # 4 samples per core
S_L = C // N_CORES            # 16 input channels per core
N_TILES = NB_L // 128         # 144 unit tiles per core
GROUP = 16                    # unit tiles per processing group
N_GROUPS = N_TILES // GROUP   # 9
BN_EPS = 1e-6
PIX = H * W                   # 1024
PAD = H + 2                   # 34

# int8 weights (host-scaled to +/-127; the uniform grid beats fp8e4m3 by
# ~8x for uniform-distributed dense_w, final rel err ~3e-3 vs the 2e-2
# budget). The on-device int8->fp16 cast is exact (integers <=127) and
# BatchNorm absorbs the host scale. fp16 z / generated kernels / x / out.
W_DT = mybir.dt.int8
Z_DT = mybir.dt.float16
K_DT = mybir.dt.float16
X_DT = mybir.dt.float16
O_DT = mybir.dt.float16

_CACHE: dict = {}


def _np_of(dt):
    return mybir.dt.np(dt)


def _build_nc(db_zero: bool, b_zero: bool):
    nc = bacc.Bacc(num_devices=N_CORES)
    f32 = mybir.dt.float32
    GC = GROUP * 128            # columns per group in unit space (2048)
    GB = GROUP * B              # columns per group in (tile,batch) space (512)

    w_in = nc.declare_dram_parameter("w", [128, 2 * NB_L], W_DT, isOutput=False)
    zT_in = nc.declare_dram_parameter("zT", [128, 2 * B], Z_DT, isOutput=False)
    gb_in = nc.declare_dram_parameter("gb_r", [128, 2, N_TILES], f32, isOutput=False)
    if not db_zero:
        db_in = nc.declare_dram_parameter("db_r", [128, N_TILES], f32, isOutput=False)
    x_in = nc.declare_dram_parameter("x", [B_L, C, PIX], X_DT, isOutput=False)
    if not b_zero:
        bv_in = nc.declare_dram_parameter("bvec", [C, 1], f32, isOutput=False)
    out_p = nc.declare_dram_parameter("out", [B_L, C, PIX], O_DT, isOutput=True)

    cc_in = nc.dram_tensor("cc_in", [B, NB_L], K_DT)
    cc_out = nc.dram_tensor("cc_out", [B, NB_L], K_DT)

    with tile.TileContext(nc) as tc:
        with tc.tile_pool(name="singles", bufs=1) as singles:
            zT = singles.tile([128, 2 * B], Z_DT)
            nc.scalar.dma_start(out=zT[:, :], in_=zT_in[:, :])
            gb = singles.tile([128, 2, N_TILES], f32)
            nc.scalar.dma_start(out=gb[:, :, :], in_=gb_in[:, :, :])
            gam = gb[:, 0, :]
            bet = gb[:, 1, :]
            if not db_zero:
                db = singles.tile([128, N_TILES], f32)
                nc.scalar.dma_start(out=db[:, :], in_=db_in[:, :])
            if not b_zero:
                bv = singles.tile([C, 1], f32)
                nc.scalar.dma_start(out=bv[:, :], in_=bv_in[:, :])
            ident = singles.tile([128, 128], K_DT)
            make_identity(nc, ident)
            eps_t = singles.tile([128, 1], f32)
            nc.vector.memset(eps_t, BN_EPS)

            # ------- Phase 1: per-group dense + BN + transpose (pipelined) ----
            with (
                tc.tile_pool(name="wpool", bufs=4) as wpool,
                tc.tile_pool(name="npool", bufs=6) as npool,
                tc.tile_pool(name="stat", bufs=6) as stat,
                tc.tile_pool(name="ph", bufs=3, space="PSUM") as psum_h,
                tc.tile_pool(name="pt", bufs=3, space="PSUM") as psum_t,
            ):
                w3 = w_in.rearrange("p (k j) -> p k j", k=2)
                last_ktr = []
                for g in range(N_GROUPS):
                    gsl = slice(g * GROUP, (g + 1) * GROUP)
                    wt8 = wpool.tile([128, 2, GC], W_DT, tag="wt8")
                    nc.sync.dma_start(
                        out=wt8[:, :, :], in_=w3[:, :, g * GC:(g + 1) * GC])
                    # exact int8 -> fp16 cast (values are integers <= 127);
                    # split across DVE and Pool so neither becomes critical
                    wt = wpool.tile([128, 2, GC], Z_DT, tag="wt")
                    nc.vector.tensor_copy(out=wt[:, 0, :], in_=wt8[:, 0, :])
                    nc.gpsimd.tensor_copy(out=wt[:, 1, :], in_=wt8[:, 1, :])
                    ps = psum_h.tile([128, GB], f32, tag="ps")
                    # write position j hosts tile t(j) = (j%4)*M4 + j//4 so
                    # the 4-tile transposes below read contiguous 128-col slabs
                    M4 = GROUP // 4
                    for j in range(GROUP):
                        t_tile = (j % 4) * M4 + j // 4
                        for k in range(2):
                            nc.tensor.matmul(
                                ps[:, j * B:(j + 1) * B],
                                wt[:, k, t_tile * 128:(t_tile + 1) * 128],
                                zT[:, k * B:(k + 1) * B],
                                start=(k == 0),
                                stop=(k == 1),
                            )
                    if not db_zero:
                        nc.vector.tensor_tensor(
                            out=ps.rearrange("p (t b) -> p t b", b=B),
                            in0=ps.rearrange("p (t b) -> p t b", b=B),
                            in1=db[:, gsl].to_broadcast([128, GROUP, B]),
                            op=mybir.AluOpType.add)
                    hg = npool.tile([128, GB], f32, tag="hg")
                    nc.scalar.activation(
                        out=hg[:, :], in_=ps[:, :],
                        func=mybir.ActivationFunctionType.Relu)
                    h3 = hg.rearrange("p (t b) -> p t b", b=B)
                    s1 = stat.tile([128, GROUP], f32, tag="s1")
                    nc.vector.reduce_sum(
                        out=s1[:, :], in_=h3, axis=mybir.AxisListType.X)
                    sq = npool.tile([128, GB], f32, tag="sq")
                    nc.vector.tensor_mul(sq[:, :], hg[:, :], hg[:, :])
                    s2 = stat.tile([128, GROUP], f32, tag="s2")
                    nc.vector.reduce_sum(
                        out=s2[:, :],
                        in_=sq.rearrange("p (t b) -> p t b", b=B),
                        axis=mybir.AxisListType.X)
                    # var = (B*S2 - S1^2)/B^2; std = sqrt(q)/B
                    t1 = stat.tile([128, GROUP], f32, tag="t1")
                    nc.scalar.activation(
                        out=t1[:, :], in_=s1[:, :],
                        func=mybir.ActivationFunctionType.Square)
                    q = stat.tile([128, GROUP], f32, tag="q")
                    nc.vector.scalar_tensor_tensor(
                        out=q[:, :], in0=s2[:, :], scalar=float(B),
                        in1=t1[:, :], op0=mybir.AluOpType.mult,
                        op1=mybir.AluOpType.subtract)
                    nc.scalar.activation(
                        out=q[:, :], in_=q[:, :],
                        func=mybir.ActivationFunctionType.Sqrt,
                        scale=float(1.0 / (B * B)))
                    nc.scalar.add(out=q[:, :], in_=q[:, :], add=eps_t[:, 0:1])
                    nc.vector.reciprocal(out=q[:, :], in_=q[:, :])
                    scl = stat.tile([128, GROUP], f32, tag="scl")
                    nc.vector.tensor_mul(scl[:, :], q[:, :], gam[:, gsl])
                    t2 = stat.tile([128, GROUP], f32, tag="t2")
                    nc.vector.tensor_mul(t2[:, :], s1[:, :], scl[:, :])
                    sft = stat.tile([128, GROUP], f32, tag="sft")
                    nc.vector.scalar_tensor_tensor(
                        out=sft[:, :], in0=t2[:, :], scalar=float(-1.0 / B),
                        in1=bet[:, gsl], op0=mybir.AluOpType.mult,
                        op1=mybir.AluOpType.add)
                    # kern = h*scale + shift, cast to K_DT (on idle GpSimd)
                    tmp = npool.tile([128, GB], f32, tag="tmp")
                    nc.gpsimd.tensor_mul(
                        tmp.rearrange("p (t b) -> p t b", b=B), h3,
                        scl[:, :].to_broadcast([128, GROUP, B]))
                    kern = npool.tile([128, GB], K_DT, tag="kern")
                    nc.gpsimd.tensor_tensor(
                        out=kern.rearrange("p (t b) -> p t b", b=B),
                        in0=tmp.rearrange("p (t b) -> p t b", b=B),
                        in1=sft[:, :].to_broadcast([128, GROUP, B]),
                        op=mybir.AluOpType.add)
                    # 4 unit-tiles per PE transpose: in (128, 4x32 strided
                    # tile-cols) -> out (128=(t4,b), 128=p)
                    pt = psum_t.tile([128, GROUP // 4 * 128], K_DT, tag="pt")
                    for m in range(GROUP // 4):
                        nc.tensor.transpose(
                            pt[:, m * 128:(m + 1) * 128],
                            kern[:, m * 128:(m + 1) * 128],
                            ident[:, :],
                        )
                    ktr = npool.tile([128, GROUP // 4 * 128], K_DT, tag="ktr")
                    nc.any.tensor_copy(out=ktr[:, :], in_=pt[:, :])
                    # dest: addr = b*NB_L + g*GC + t4*(M4*128) + (m*128 + p)
                    for t4 in range(4):
                        dst = bass.AP(
                            tensor=cc_in[:, :].tensor,
                            offset=g * GC + t4 * (M4 * 128),
                            ap=[[NB_L, B], [1, M4 * 128]],
                        )
                        eng = (nc.scalar, nc.gpsimd)[(g * 4 + t4) % 2]
                        kd = eng.dma_start(
                            out=dst, in_=ktr[t4 * B:(t4 + 1) * B, :])
                        if g == N_GROUPS - 1:
                            last_ktr.append(kd)

            # ---------------- Phase 2 prep (overlaps the collective) ---------
            with (
                tc.tile_pool(name="conv", bufs=1) as conv,
                tc.tile_pool(name="conv2", bufs=2) as conv2,
                tc.tile_pool(name="po", bufs=4, space="PSUM") as psum_o,
            ):
                # ---------------- AllToAll: batch redistribute ---------------
                nc.gpsimd.collective_compute(
                    "AllToAll",
                    mybir.AluOpType.bypass,
                    replica_groups=[list(range(N_CORES))],
                    ins=[cc_in[:, :]],
                    outs=[cc_out[:, :]],
                )

                # phase-2 prep: deferred until phase-1 stores finish so it
                # fills the collective window instead of stalling the phase-1
                # tail; kept off gpsimd so the A2A issue is never queued
                # behind it on the Pool sequencer
                x_all = conv.tile([128, B_L, PIX], X_DT)
                xd = nc.scalar.dma_start(
                    out=x_all[:, :, :],
                    in_=x_in.rearrange("b p j -> p b j"))
                add_dep_helper(xd.ins, last_ktr[-1].ins, sync=True,
                               reason="defer x load into A2A window")
                xp_all = conv.tile([128, B_L, PAD * PAD], X_DT)
                ms = nc.vector.memset(xp_all[:, :, :], 0.0)
                add_dep_helper(ms.ins, last_ktr[-1].ins, sync=True,
                               reason="defer pad memset into A2A window")
                nc.vector.tensor_copy(
                    out=xp_all.rearrange("p b (r c) -> p b r c", c=PAD)
                    [:, :, 1:H + 1, 1:W + 1],
                    in_=x_all.rearrange("p b (r c) -> p b r c", c=W),
                )

                # ---------------- Phase 2: per-sample conv + residual --------
                hks = []
                for i in range(B_L):
                    hk = conv2.tile([128, C * KH * KW], K_DT, tag="hk")
                    if i == 0:
                        # split the first load across two engines: it gates
                        # the whole conv phase right after the AllToAll
                        for hf in range(2):
                            srcap = bass.AP(
                                tensor=cc_out[:, :].tensor,
                                offset=i * NB_L + hf * 4 * B_L * NB_L,
                                ap=[[B_L * NB_L, N_CORES // 2],
                                    [C * KH * KW, S_L], [1, C * KH * KW]],
                            )
                            (nc.scalar, nc.sync)[hf].dma_start(
                                out=hk[hf * 64:(hf + 1) * 64, :], in_=srcap)
                    else:
                        srcap = bass.AP(
                            tensor=cc_out[:, :].tensor,
                            offset=i * NB_L,
                            ap=[[B_L * NB_L, N_CORES], [C * KH * KW, S_L],
                                [1, C * KH * KW]],
                        )
                        (nc.scalar, nc.sync)[i % 2].dma_start(
                            out=hk[:, :], in_=srcap)
                    hks.append(hk)
                for i in range(B_L):
                    po = psum_o.tile([128, PIX], f32, tag="po")
                    hk9 = hks[i].rearrange("p (f n) -> p n f", n=KH * KW)
                    xp3 = xp_all[:, i, :].rearrange("p (r c) -> p r c", c=PAD)
                    ob = conv2.tile([128, PIX], O_DT, tag="ob")
                    last = i == B_L - 1
                    for hh in range(2):
                        for tap in range(KH * KW):
                            u, v = tap // KW, tap % KW
                            r0 = hh * 16 + u
                            nc.tensor.matmul(
                                po[:, hh * 512:(hh + 1) * 512],
                                hk9[:, tap, :],
                                xp3[:, r0:r0 + 16, v:v + W],
                                start=(tap == 0),
                                stop=(tap == KH * KW - 1),
                            )
                        if last:
                            # epilogue per half, inline: half-0's add+store
                            # overlaps half-1's taps on the PE
                            hsl = slice(hh * 512, (hh + 1) * 512)
                            nc.vector.tensor_add(
                                out=ob[:, hsl], in0=po[:, hsl],
                                in1=x_all[:, i, hsl])
                            if not b_zero:
                                nc.scalar.add(
                                    out=ob[:, hsl], in_=ob[:, hsl],
                                    add=bv[:, 0:1])
                            nc.sync.dma_start(
                                out=out_p[i, :, hsl], in_=ob[:, hsl])
                    if not last:
                        # out = conv + x (+ b)
                        nc.vector.tensor_add(
                            out=ob[:, :], in0=po[:, :], in1=x_all[:, i, :])
                        if not b_zero:
                            nc.scalar.add(
                                out=ob[:, :], in_=ob[:, :], add=bv[:, 0:1])
                        nc.sync.dma_start(
                            out=out_p[i, :, :], in_=ob[:, :])

    nc.compile()
    return nc


def _perm_groups(v144):
    # reorder unit-tile columns so position j = m*4+t4 holds tile t4*(G/4)+m
    r = v144.reshape(128, N_TILES // GROUP, 4, GROUP // 4)
    return np.ascontiguousarray(r.transpose(0, 1, 3, 2).reshape(128, N_TILES))


def _make_in_maps(x, z, dense_w, dense_b, gamma, beta, b):
    znp = _np_of(Z_DT)
    f32 = np.float32
    # zT[p, k*B + bb] = z[bb, 128k + p]
    zr = np.ascontiguousarray(z.T.astype(f32)).reshape(2, 128, B)
    zT = np.concatenate([zr[0], zr[1]], axis=1).astype(znp)
    bvec = np.asarray(b, dtype=f32).reshape(C, 1)
    wf = np.asarray(dense_w, dtype=f32)
    wmax = float(np.abs(wf).max())
    alpha = f32(127.0 / wmax) if wmax > 0 else f32(1.0)
    ws_all = np.rint(wf * alpha)
    in_maps = []
    for c in range(N_CORES):
        sl = slice(c * NB_L, (c + 1) * NB_L)
        ws = ws_all[:, sl]
        w_host = np.ascontiguousarray(
            np.concatenate([ws[:128, :], ws[128:, :]], axis=1)).astype(np.int8)
        gb_r = np.stack([
            _perm_groups(
                np.asarray(gamma, dtype=f32)[sl].reshape(N_TILES, 128).T),
            _perm_groups(
                np.asarray(beta, dtype=f32)[sl].reshape(N_TILES, 128).T),
        ], axis=1)
        in_maps.append({
            "w": w_host,
            "zT": zT,
            "gb_r": np.ascontiguousarray(gb_r),
            # dense_b rides the scaled pre-activation: scale it to match
            "db_r": _perm_groups(
                (np.asarray(dense_b, dtype=f32) * alpha)[sl]
                .reshape(N_TILES, 128).T),
            "x": np.ascontiguousarray(
                np.asarray(x, dtype=f32)[c * B_L:(c + 1) * B_L]
                .reshape(B_L, C, PIX)).astype(_np_of(X_DT)),
            "bvec": bvec,
        })
    return in_maps


def kernel(x, z, dense_w, dense_b, gamma, beta, b):
    import time

    x, z, dense_w = np.asarray(x), np.asarray(z), np.asarray(dense_w)
    dense_b, gamma = np.asarray(dense_b), np.asarray(gamma)
    beta, b = np.asarray(beta), np.asarray(b)
    key = (bool(np.all(dense_b == 0)), bool(np.all(b == 0)))
    if key not in _CACHE:
        _CACHE[key] = _build_nc(*key)
        _CACHE["nc"] = _CACHE[key]
    nc = _CACHE[key]
    db_zero, b_zero = key
    in_maps = _make_in_maps(x, z, dense_w, dense_b, gamma, beta, b)
    for m in in_maps:
        if db_zero:
            m.pop("db_r", None)
        if b_zero:
            m.pop("bvec", None)
    res = None
    for attempt in range(3):
        try:
            res = run_bass_kernel_spmd(nc, in_maps, list(range(N_CORES)))
            break
        except Exception:
            # transient NRT device-unrecoverable errors heal on retry
            if attempt == 2:
                raise
            time.sleep(2.0)
    out = np.concatenate(
        [res.results[c]["out"].reshape(B_L, C, H, W) for c in range(N_CORES)],
        axis=0,
    )
    return out.astype(np.float32)


# revision 29
# speedup vs baseline: 2.1593x; 1.7909x over previous
"""Trainium2 Bass kernel for nn_Deconv2DVariableWeights (8-core SPMD).

Strategy:
  Phase 1 (dense + BN), unit-sharded: core c owns dense_w columns
  [c*18432, (c+1)*18432) = conv input channels s in [16c, 16c+16).
  It computes h = relu(z @ w + b) for ALL 32 samples in (units, batch)
  layout, so BatchNorm batch statistics are local free-dim reductions.
  Normalized kernels are PE-transposed to (batch, units) rows and
  written to DRAM.

  AllToAll redistributes kernels: afterwards core c holds the FULL
  147456-long kernel vector for its 4 samples (batch-sharded).

  Phase 2 (conv + residual), batch-sharded: per sample, the 3x3
  conv is 9 PSUM-accumulated matmuls (K=s=128, M=f=128, N=512x2)
  against a zero-padded image tile addressed with strided APs.

  Dispatch-IO precision: the per-dispatch cost through the axon tunnel
  is dominated by staging the argument buffers (~360 GB/s aggregate),
  so the kernel minimizes IO bytes. dense_w ships as int8 (host-scaled
  to +/-127; the scale is absorbed exactly by the BatchNorm divide and
  the on-device int8->fp16 cast is exact). x and out ship as fp16;
  gamma/beta ride in one stacked tensor.
"""

import numpy as np

import concourse.bacc as bacc
import concourse.bass as bass
import concourse.tile as tile
from concourse import mybir
from concourse.bass_utils import run_bass_kernel_spmd
from concourse.masks import make_identity
from concourse.tile import add_dep_helper

# Problem constants (hardcoded per contract)
B, C, H, W = 32, 128, 32, 32
ZDIM = 256
KH = KW = 3
NB = C * C * KH * KW          # 147456
N_CORES = 8
NB_L = NB // N_CORES          # 18432 units per core
B_L = B // N_CORES            # 4 samples per core
S_L = C // N_CORES            # 16 input channels per core
N_TILES = NB_L // 128         # 144 unit tiles per core
GROUP = 16                    # unit tiles per processing group
N_GROUPS = N_TILES // GROUP   # 9
BN_EPS = 1e-6
PIX = H * W                   # 1024
PAD = H + 2                   # 34

# int8 weights (host-scaled to +/-127; the uniform grid beats fp8e4m3 by
# ~8x for uniform-distributed dense_w, final rel err ~3e-3 vs the 2e-2
# budget). The on-device int8->fp16 cast is exact (integers <=127) and
# BatchNorm absorbs the host scale. fp16 z / generated kernels / x / out.
W_DT = mybir.dt.int8
Z_DT = mybir.dt.float16
K_DT = mybir.dt.float16
X_DT = mybir.dt.float16
O_DT = mybir.dt.float16

_CACHE: dict = {}


def _np_of(dt):
    return mybir.dt.np(dt)


def _build_nc(db_zero: bool, b_zero: bool):
    nc = bacc.Bacc(num_devices=N_CORES)
    f32 = mybir.dt.float32
    GC = GROUP * 128            # columns per group in unit space (2048)
    GB = GROUP * B              # columns per group in (tile,batch) space (512)

    # all inputs ride ONE packed int8 tensor: per-dispatch cost through the
    # axon tunnel has a large per-argument component, so fewer args win.
    # layout per partition p (byte offsets):
    #   [0, 36864)          w int8, k-half-concat layout
    #   [36864, 36992)      zT fp16 (64 values)
    #   [36992, 37568)      gamma/beta fp16 (2 x 144 values)
    #   [37568, 45760)      x fp16 (B_L x PIX values, partition = channel)
    MEGA_ZT = 2 * NB_L
    MEGA_GB = MEGA_ZT + 2 * (2 * B)
    MEGA_X = MEGA_GB + 2 * (2 * N_TILES)
    MEGA_COLS = MEGA_X + 2 * (B_L * PIX)
    mega_in = nc.declare_dram_parameter(
        "mega", [128, MEGA_COLS], mybir.dt.int8, isOutput=False)
    if not db_zero:
        db_in = nc.declare_dram_parameter("db_r", [128, N_TILES], f32, isOutput=False)
    if not b_zero:
        bv_in = nc.declare_dram_parameter("bvec", [C, 1], f32, isOutput=False)
    out_p = nc.declare_dram_parameter("out", [B_L, C, PIX], O_DT, isOutput=True)

    cc_in = nc.dram_tensor("cc_in", [B, NB_L], K_DT)
    cc_out = nc.dram_tensor("cc_out", [B, NB_L], K_DT)

    with tile.TileContext(nc) as tc:
        with tc.tile_pool(name="singles", bufs=1) as singles:
            zT = singles.tile([128, 2 * B], Z_DT)
            nc.scalar.dma_start(
                out=zT[:, :], in_=mega_in[:, MEGA_ZT:MEGA_GB].bitcast(Z_DT))
            gb = singles.tile([128, 2, N_TILES], Z_DT)
            nc.scalar.dma_start(
                out=gb[:, :, :],
                in_=mega_in[:, MEGA_GB:MEGA_X].bitcast(Z_DT)
                .rearrange("p (k t) -> p k t", k=2))
            gam = gb[:, 0, :]
            bet = gb[:, 1, :]
            if not db_zero:
                db = singles.tile([128, N_TILES], f32)
                nc.scalar.dma_start(out=db[:, :], in_=db_in[:, :])
            if not b_zero:
                bv = singles.tile([C, 1], f32)
                nc.scalar.dma_start(out=bv[:, :], in_=bv_in[:, :])
            ident = singles.tile([128, 128], K_DT)
            make_identity(nc, ident)
            eps_t = singles.tile([128, 1], f32)
            nc.vector.memset(eps_t, BN_EPS)

            # ------- Phase 1: per-group dense + BN + transpose (pipelined) ----
            with (
                tc.tile_pool(name="wpool", bufs=4) as wpool,
                tc.tile_pool(name="npool", bufs=8) as npool,
                tc.tile_pool(name="stat", bufs=6) as stat,
                tc.tile_pool(name="ph", bufs=4, space="PSUM") as psum_h,
                tc.tile_pool(name="pt", bufs=3, space="PSUM") as psum_t,
            ):
                w3 = mega_in[:, 0:MEGA_ZT].rearrange("p (k j) -> p k j", k=2)
                last_ktr = []
                for g in range(N_GROUPS):
                    gsl = slice(g * GROUP, (g + 1) * GROUP)
                    wt8 = wpool.tile([128, 2, GC], W_DT, tag="wt8")
                    nc.sync.dma_start(
                        out=wt8[:, :, :], in_=w3[:, :, g * GC:(g + 1) * GC])
                    # exact int8 -> fp16 cast (values are integers <= 127);
                    # split three ways so no single engine owns the cost
                    wt = wpool.tile([128, 2, GC], Z_DT, tag="wt")
                    w2f = wt.rearrange("p k j -> p (k j)")
                    w28 = wt8.rearrange("p k j -> p (k j)")
                    nc.vector.tensor_copy(
                        out=w2f[:, 0:1536], in_=w28[:, 0:1536])
                    nc.gpsimd.tensor_copy(
                        out=w2f[:, 1536:3072], in_=w28[:, 1536:3072])
                    nc.scalar.copy(
                        out=w2f[:, 3072:2 * GC], in_=w28[:, 3072:2 * GC])
                    ps = psum_h.tile([128, GB], f32, tag="ps")
                    # write position j hosts tile t(j) = (j%4)*M4 + j//4 so
                    # the 4-tile transposes below read contiguous 128-col slabs
                    M4 = GROUP // 4
                    for j in range(GROUP):
                        t_tile = (j % 4) * M4 + j // 4
                        for k in range(2):
                            nc.tensor.matmul(
                                ps[:, j * B:(j + 1) * B],
                                wt[:, k, t_tile * 128:(t_tile + 1) * 128],
                                zT[:, k * B:(k + 1) * B],
                                start=(k == 0),
                                stop=(k == 1),
                            )
                    if not db_zero:
                        nc.vector.tensor_tensor(
                            out=ps.rearrange("p (t b) -> p t b", b=B),
                            in0=ps.rearrange("p (t b) -> p t b", b=B),
                            in1=db[:, gsl].to_broadcast([128, GROUP, B]),
                            op=mybir.AluOpType.add)
                    # relu output in fp16 at 2^-6 scale: keeps h and h^2 in
                    # fp16 range (BN absorbs the scale exactly) and doubles
                    # DVE throughput for the stat passes
                    hg = npool.tile([128, GB], Z_DT, tag="hg")
                    nc.scalar.activation(
                        out=hg[:, :], in_=ps[:, :],
                        func=mybir.ActivationFunctionType.Relu,
                        scale=float(2.0 ** -6))
                    h3 = hg.rearrange("p (t b) -> p t b", b=B)
                    s1 = stat.tile([128, GROUP], f32, tag="s1")
                    nc.vector.reduce_sum(
                        out=s1[:, :], in_=h3, axis=mybir.AxisListType.X)
                    sq = npool.tile([128, GB], Z_DT, tag="sq")
                    nc.vector.tensor_mul(sq[:, :], hg[:, :], hg[:, :])
                    s2 = stat.tile([128, GROUP], f32, tag="s2")
                    nc.vector.reduce_sum(
                        out=s2[:, :],
                        in_=sq.rearrange("p (t b) -> p t b", b=B),
                        axis=mybir.AxisListType.X)
                    # var = (B*S2 - S1^2)/B^2; std = sqrt(q)/B
                    t1 = stat.tile([128, GROUP], f32, tag="t1")
                    nc.scalar.activation(
                        out=t1[:, :], in_=s1[:, :],
                        func=mybir.ActivationFunctionType.Square)
                    q = stat.tile([128, GROUP], f32, tag="q")
                    nc.vector.scalar_tensor_tensor(
                        out=q[:, :], in0=s2[:, :], scalar=float(B),
                        in1=t1[:, :], op0=mybir.AluOpType.mult,
                        op1=mybir.AluOpType.subtract)
                    nc.scalar.activation(
                        out=q[:, :], in_=q[:, :],
                        func=mybir.ActivationFunctionType.Sqrt,
                        scale=float(1.0 / (B * B)))
                    nc.scalar.add(out=q[:, :], in_=q[:, :], add=eps_t[:, 0:1])
                    nc.vector.reciprocal(out=q[:, :], in_=q[:, :])
                    scl = stat.tile([128, GROUP], f32, tag="scl")
                    nc.vector.tensor_mul(scl[:, :], q[:, :], gam[:, gsl])
                    t2 = stat.tile([128, GROUP], f32, tag="t2")
                    nc.vector.tensor_mul(t2[:, :], s1[:, :], scl[:, :])
                    sft = stat.tile([128, GROUP], f32, tag="sft")
                    nc.vector.scalar_tensor_tensor(
                        out=sft[:, :], in0=t2[:, :], scalar=float(-1.0 / B),
                        in1=bet[:, gsl], op0=mybir.AluOpType.mult,
                        op1=mybir.AluOpType.add)
                    # kern = h*scale + shift, cast to K_DT (on idle GpSimd)
                    tmp = npool.tile([128, GB], Z_DT, tag="tmp")
                    nc.gpsimd.tensor_mul(
                        tmp.rearrange("p (t b) -> p t b", b=B), h3,
                        scl[:, :].to_broadcast([128, GROUP, B]))
                    kern = npool.tile([128, GB], K_DT, tag="kern")
                    nc.gpsimd.tensor_tensor(
                        out=kern.rearrange("p (t b) -> p t b", b=B),
                        in0=tmp.rearrange("p (t b) -> p t b", b=B),
                        in1=sft[:, :].to_broadcast([128, GROUP, B]),
                        op=mybir.AluOpType.add)
                    # 4 unit-tiles per PE transpose: in (128, 4x32 strided
                    # tile-cols) -> out (128=(t4,b), 128=p)
                    pt = psum_t.tile([128, GROUP // 4 * 128], K_DT, tag="pt")
                    for m in range(GROUP // 4):
                        nc.tensor.transpose(
                            pt[:, m * 128:(m + 1) * 128],
                            kern[:, m * 128:(m + 1) * 128],
                            ident[:, :],
                        )
                    ktr = npool.tile([128, GROUP // 4 * 128], K_DT, tag="ktr")
                    nc.any.tensor_copy(out=ktr[:, :], in_=pt[:, :])
                    # dest: addr = b*NB_L + g*GC + t4*(M4*128) + (m*128 + p)
                    for t4 in range(4):
                        dst = bass.AP(
                            tensor=cc_in[:, :].tensor,
                            offset=g * GC + t4 * (M4 * 128),
                            ap=[[NB_L, B], [1, M4 * 128]],
                        )
                        eng = (nc.scalar, nc.gpsimd, nc.sync)[(g * 4 + t4) % 3]
                        kd = eng.dma_start(
                            out=dst, in_=ktr[t4 * B:(t4 + 1) * B, :])
                        if g == N_GROUPS - 1:
                            last_ktr.append(kd)

            # ---------------- Phase 2 prep (overlaps the collective) ---------
            with (
                tc.tile_pool(name="conv", bufs=1) as conv,
                tc.tile_pool(name="conv2", bufs=2) as conv2,
                tc.tile_pool(name="po", bufs=4, space="PSUM") as psum_o,
            ):
                # ---------------- AllToAll: batch redistribute ---------------
                nc.gpsimd.collective_compute(
                    "AllToAll",
                    mybir.AluOpType.bypass,
                    replica_groups=[list(range(N_CORES))],
                    ins=[cc_in[:, :]],
                    outs=[cc_out[:, :]],
                )

                # phase-2 prep: deferred until phase-1 stores finish so it
                # fills the collective window instead of stalling the phase-1
                # tail; kept off gpsimd so the A2A issue is never queued
                # behind it on the Pool sequencer
                x_all = conv.tile([128, B_L, PIX], X_DT)
                xd = nc.scalar.dma_start(
                    out=x_all[:, :, :],
                    in_=mega_in[:, MEGA_X:MEGA_COLS].bitcast(X_DT)
                    .rearrange("p (b j) -> p b j", b=B_L))
                add_dep_helper(xd.ins, last_ktr[-1].ins, sync=True,
                               reason="defer x load into A2A window")
                xp_all = conv.tile([128, B_L, PAD * PAD], X_DT)
                ms = nc.vector.memset(xp_all[:, :, :], 0.0)
                add_dep_helper(ms.ins, last_ktr[-1].ins, sync=True,
                               reason="defer pad memset into A2A window")
                nc.vector.tensor_copy(
                    out=xp_all.rearrange("p b (r c) -> p b r c", c=PAD)
                    [:, :, 1:H + 1, 1:W + 1],
                    in_=x_all.rearrange("p b (r c) -> p b r c", c=W),
                )

                # ---------------- Phase 2: per-sample conv + residual --------
                hks = []
                for i in range(B_L):
                    hk = conv2.tile([128, C * KH * KW], K_DT, tag="hk")
                    if i == 0:
                        # split the first load across two engines: it gates
                        # the whole conv phase right after the AllToAll
                        for hf in range(2):
                            srcap = bass.AP(
                                tensor=cc_out[:, :].tensor,
                                offset=i * NB_L + hf * 4 * B_L * NB_L,
                                ap=[[B_L * NB_L, N_CORES // 2],
                                    [C * KH * KW, S_L], [1, C * KH * KW]],
                            )
                            (nc.scalar, nc.sync)[hf].dma_start(
                                out=hk[hf * 64:(hf + 1) * 64, :], in_=srcap)
                    else:
                        srcap = bass.AP(
                            tensor=cc_out[:, :].tensor,
                            offset=i * NB_L,
                            ap=[[B_L * NB_L, N_CORES], [C * KH * KW, S_L],
                                [1, C * KH * KW]],
                        )
                        (nc.scalar, nc.sync)[i % 2].dma_start(
                            out=hk[:, :], in_=srcap)
                    hks.append(hk)
                for i in range(B_L):
                    po = psum_o.tile([128, PIX], f32, tag="po")
                    hk9 = hks[i].rearrange("p (f n) -> p n f", n=KH * KW)
                    xp3 = xp_all[:, i, :].rearrange("p (r c) -> p r c", c=PAD)
                    ob = conv2.tile([128, PIX], O_DT, tag="ob")
                    last = i == B_L - 1
                    for hh in range(2):
                        for tap in range(KH * KW):
                            u, v = tap // KW, tap % KW
                            r0 = hh * 16 + u
                            nc.tensor.matmul(
                                po[:, hh * 512:(hh + 1) * 512],
                                hk9[:, tap, :],
                                xp3[:, r0:r0 + 16, v:v + W],
                                start=(tap == 0),
                                stop=(tap == KH * KW - 1),
                            )
                        if last:
                            # epilogue per half, inline: half-0's add+store
                            # overlaps half-1's taps on the PE
                            hsl = slice(hh * 512, (hh + 1) * 512)
                            nc.vector.tensor_add(
                                out=ob[:, hsl], in0=po[:, hsl],
                                in1=x_all[:, i, hsl])
                            if not b_zero:
                                nc.scalar.add(
                                    out=ob[:, hsl], in_=ob[:, hsl],
                                    add=bv[:, 0:1])
                            nc.sync.dma_start(
                                out=out_p[i, :, hsl], in_=ob[:, hsl])
                    if not last:
                        # out = conv + x (+ b)
                        nc.vector.tensor_add(
                            out=ob[:, :], in0=po[:, :], in1=x_all[:, i, :])
                        if not b_zero:
                            nc.scalar.add(
                                out=ob[:, :], in_=ob[:, :], add=bv[:, 0:1])
                        nc.sync.dma_start(
                            out=out_p[i, :, :], in_=ob[:, :])

    nc.compile()
    return nc


def _perm_groups(v144):
    # reorder unit-tile columns so position j = m*4+t4 holds tile t4*(G/4)+m
    r = v144.reshape(128, N_TILES // GROUP, 4, GROUP // 4)
    return np.ascontiguousarray(r.transpose(0, 1, 3, 2).reshape(128, N_TILES))


def _make_in_maps(x, z, dense_w, dense_b, gamma, beta, b):
    znp = _np_of(Z_DT)
    f32 = np.float32
    # zT[p, k*B + bb] = z[bb, 128k + p]
    zr = np.ascontiguousarray(z.T.astype(f32)).reshape(2, 128, B)
    zT = np.concatenate([zr[0], zr[1]], axis=1).astype(znp)
    bvec = np.asarray(b, dtype=f32).reshape(C, 1)
    wf = np.asarray(dense_w, dtype=f32)
    wmax = float(np.abs(wf).max())
    alpha = f32(127.0 / wmax) if wmax > 0 else f32(1.0)
    ws_all = np.rint(wf * alpha)
    zT_bytes = np.ascontiguousarray(zT).view(np.int8)
    in_maps = []
    for c in range(N_CORES):
        sl = slice(c * NB_L, (c + 1) * NB_L)
        ws = ws_all[:, sl]
        w_host = np.ascontiguousarray(
            np.concatenate([ws[:128, :], ws[128:, :]], axis=1)).astype(np.int8)
        gb_r = np.stack([
            _perm_groups(
                np.asarray(gamma, dtype=f32)[sl].reshape(N_TILES, 128).T),
            _perm_groups(
                np.asarray(beta, dtype=f32)[sl].reshape(N_TILES, 128).T),
        ], axis=1).astype(znp)
        gb_bytes = np.ascontiguousarray(gb_r).reshape(128, -1).view(np.int8)
        # x packed partition(channel)-major: [C, B_L*PIX] fp16
        xc = (np.asarray(x, dtype=f32)[c * B_L:(c + 1) * B_L]
              .reshape(B_L, C, PIX).transpose(1, 0, 2).astype(_np_of(X_DT)))
        x_bytes = np.ascontiguousarray(xc).reshape(128, -1).view(np.int8)
        mega = np.ascontiguousarray(np.concatenate(
            [w_host, zT_bytes, gb_bytes, x_bytes], axis=1))
        in_maps.append({
            "mega": mega,
            # dense_b rides the scaled pre-activation: scale it to match
            "db_r": _perm_groups(
                (np.asarray(dense_b, dtype=f32) * alpha)[sl]
                .reshape(N_TILES, 128).T),
            "bvec": bvec,
        })
    return in_maps


def kernel(x, z, dense_w, dense_b, gamma, beta, b):
    import time

    x, z, dense_w = np.asarray(x), np.asarray(z), np.asarray(dense_w)
    dense_b, gamma = np.asarray(dense_b), np.asarray(gamma)
    beta, b = np.asarray(beta), np.asarray(b)
    key = (bool(np.all(dense_b == 0)), bool(np.all(b == 0)))
    if key not in _CACHE:
        _CACHE[key] = _build_nc(*key)
        _CACHE["nc"] = _CACHE[key]
    nc = _CACHE[key]
    db_zero, b_zero = key
    in_maps = _make_in_maps(x, z, dense_w, dense_b, gamma, beta, b)
    for m in in_maps:
        if db_zero:
            m.pop("db_r", None)
        if b_zero:
            m.pop("bvec", None)
    res = None
    for attempt in range(3):
        try:
            res = run_bass_kernel_spmd(nc, in_maps, list(range(N_CORES)))
            break
        except Exception:
            # transient NRT device-unrecoverable errors heal on retry
            if attempt == 2:
                raise
            time.sleep(2.0)
    out = np.concatenate(
        [res.results[c]["out"].reshape(B_L, C, H, W) for c in range(N_CORES)],
        axis=0,
    )
    return out.astype(np.float32)
